# revision 54
# baseline (speedup 1.0000x reference)
"""TRN2 Bass kernel for nn_CausalSelfAttention_4054449128214.

The reference returns out_s + stop_gradient(out_full - out_s), whose forward
value is exactly out_full -- plain dense causal self-attention. So the kernel
computes: qkv = x@W_attn+b_attn, per-head causal softmax attention,
y@W_proj+b_proj.

Sharding (8 cores, no collectives): Megatron head-parallel. Cores 0-3 own head
pairs (0,1)..(6,7); cores 4-7 own heads 8..11 (duplicated for SPMD
shape-uniformity, second W_proj slice zeroed). Each core computes its heads'
QKV columns, attention, and a row-sliced partial output projection; the host
sums the 8 partials (the Megatron row-parallel all-reduce) and transposes.

Design (36.7us cost-model time vs 46.2us f32r baseline):
- everything bf16: halves DMA, full-rate matmuls at any free size, 2x DVE
  modes. End-to-end rel err ~4e-3 (gate 2e-2).
- V is produced directly transposed ([token, channel]) by operand-swapped
  matmuls (stationary = x chunk, moving = W_v); the softmax denominator rides
  as an all-ones 65th column of the V^T tiles, so it falls out of the AV
  matmul for free.
- causal skip: upper-diagonal key chunks run on half (256-query) tiles, and
  mask multiplies only touch the staircase region (<=256 cols) of each e
  tile; fully-visible columns never wait on a mask.
- output projection uses contraction 128 (both heads in one matmul);
  queries 512:768 are normalized/projected as soon as key chunk 5 lands
  (the later chunks only touch columns 256:512 of the PSUM accumulator).
- engine roles: Act = exp stream (+ the PSUM->SBUF copies that gate it,
  which its in-order queue naturally sequences); DVE = masks, normalize,
  most proj copies; Pool = partition broadcasts, late-half affine masks,
  memsets; PE warms its p-state on a dummy chain during the input DMAs.
- PSUM: one 6-deep [128,512] ring (qkv/scores/vT/proj) + 2 banks for the
  two per-head [65,512] AV accumulators.
- PSUM start=True zeroes a whole 2KB zero-region, DMA cannot touch PSUM,
  GPSIMD cannot touch PSUM, and TensorTensor may read only one PSUM
  operand -- all load-bearing constraints below.
"""

import numpy as np
import ml_dtypes

import concourse.bacc as bacc
import concourse.mybir as mybir
import concourse.tile as tile
from concourse.bass_utils import run_bass_kernel_spmd

F32 = mybir.dt.float32
BF16 = mybir.dt.bfloat16
AF = mybir.ActivationFunctionType

T = 1024          # sequence length
C = 768           # channels
NH = 12           # heads
HS = 64           # head size
NCORES = 8
TT = 512          # query tile
HT = 256          # half tile
NCC = C // 128    # 6 contraction chunks
NKC = T // 128    # 8 key chunks
SCALE = 1.0 / 8.0  # 1/sqrt(HS)

# core -> (head0, head1); cores 4-7 duplicate their head (2nd W_proj slice zeroed)
HEAD_MAP = [(0, 1), (2, 3), (4, 5), (6, 7), (8, 8), (9, 9), (10, 10), (11, 11)]

_CACHE: dict = {}
_DBG: dict = {}


def _build_program():
    nc = bacc.Bacc("TRN2", target_bir_lowering=False, debug=False,
                   num_devices=NCORES)
    xT = nc.dram_tensor("xT", [C, T], BF16, kind="ExternalInput").ap()
    wq = nc.dram_tensor("wq", [C, 128], BF16, kind="ExternalInput").ap()
    wk = nc.dram_tensor("wk", [C, 128], BF16, kind="ExternalInput").ap()
    wv = nc.dram_tensor("wv", [C, 128], BF16, kind="ExternalInput").ap()
    wp = nc.dram_tensor("wp", [128, C], BF16, kind="ExternalInput").ap()
    bqk = nc.dram_tensor("bqk", [128, 2], F32, kind="ExternalInput").ap()
    bpr = nc.dram_tensor("bpr", [128, NCC], F32, kind="ExternalInput").ap()
    outT = nc.dram_tensor("outT", [C, T], BF16, kind="ExternalOutput").ap()

    with tile.TileContext(nc) as tc:
        with (
            tc.tile_pool(name="const", bufs=1) as cp,
            tc.tile_pool(name="e", bufs=8) as ep,
            tc.tile_pool(name="rb", bufs=2) as rbp,
            tc.tile_pool(name="pst", bufs=4, space="PSUM") as pst,
            tc.tile_pool(name="pov", bufs=2, space="PSUM") as pov,
        ):
            # preload the Act function table (exp_and_others) and ramp the
            # PE p-state off the critical path while DMAs run
            tiny = cp.tile([1, 128], BF16, tag="tiny")
            nc.vector.memset(tiny[:], 1.0)
            nc.scalar.activation(tiny[:, 64:65], tiny[:, 0:1], AF.Exp)
            pwu = pst.tile([128, TT], F32, tag="st")
            for _ in range(45):
                nc.tensor.matmul(pwu[0:1, 0:64], tiny[:, 0:1], tiny[:, 0:64],
                                 start=True, stop=True)

            # ---- input DMAs (SP queue), in need order
            wqt = cp.tile([128, NCC * 128], BF16, tag="wqt")
            nc.sync.dma_start(
                out=wqt[:].rearrange("p (c j) -> p c j", c=NCC),
                in_=wq.rearrange("(c p) j -> p c j", p=128))
            xt = cp.tile([128, NCC * T], BF16, tag="xt")
            x3 = xT.rearrange("(c p) t -> p c t", p=128)
            xv = xt[:].rearrange("p (c t) -> p c t", c=NCC)
            for c0 in (0, 2):             # first 512 tokens, cc pairs
                nc.sync.dma_start(out=xv[:, c0:c0 + 2, 0:TT],
                                  in_=x3[:, c0:c0 + 2, 0:TT])
            wkt = cp.tile([128, NCC * 128], BF16, tag="wkt")
            nc.sync.dma_start(
                out=wkt[:].rearrange("p (c j) -> p c j", c=NCC),
                in_=wk.rearrange("(c p) j -> p c j", p=128))
            nc.sync.dma_start(out=xv[:, 4:6, 0:TT], in_=x3[:, 4:6, 0:TT])
            wvt = cp.tile([128, NCC * 128], BF16, tag="wvt")
            nc.sync.dma_start(
                out=wvt[:].rearrange("p (c j) -> p c j", c=NCC),
                in_=wv.rearrange("(c p) j -> p c j", p=128))
            for c0 in (0, 2, 4):          # second 512 tokens
                nc.sync.dma_start(out=xv[:, c0:c0 + 2, TT:T],
                                  in_=x3[:, c0:c0 + 2, TT:T])
            bpr_sb = cp.tile([128, NCC], F32, tag="bpr")
            nc.sync.dma_start(out=bpr_sb[:], in_=bpr)
            wpt = cp.tile([128, C], BF16, tag="wpt")
            nc.sync.dma_start(out=wpt[:], in_=wp)

            # tiny early constant on Pool's DMA queue
            bqk_sb = cp.tile([128, 2], F32, tag="bqk")
            nc.gpsimd.dma_start(out=bqk_sb[:], in_=bqk)


            ones64 = cp.tile([1, 64], BF16, tag="ones64")
            nc.gpsimd.memset(ones64[:], 1.0)

            # causal staircase masks, generated on the idle early Pool engine:
            # cols 0:256 keep q>=p (M0), cols 256:512 keep q>=p+128 (M1)
            masks = cp.tile([128, TT], BF16, tag="masks")
            nc.gpsimd.memset(masks[:], 1.0)
            nc.gpsimd.affine_select(
                masks[:, 0:HT], masks[:, 0:HT], pattern=[[1, HT]],
                compare_op=mybir.AluOpType.is_ge, fill=0.0,
                base=0, channel_multiplier=-1)
            nc.gpsimd.affine_select(
                masks[:, HT:TT], masks[:, HT:TT], pattern=[[1, HT]],
                compare_op=mybir.AluOpType.is_ge, fill=0.0,
                base=-128, channel_multiplier=-1)

            # V^T tiles: [key, 2*(64 ch + ones col)]; ones preset via memset
            vaug = [cp.tile([128, 130], BF16, tag=f"va{kc}", name=f"va{kc}")
                    for kc in range(NKC)]
            for kc in range(NKC):
                nc.gpsimd.memset(vaug[kc][:], 1.0)

            ws_q = [wqt[:, cc * 128:(cc + 1) * 128] for cc in range(NCC)]
            ws_k = [wkt[:, cc * 128:(cc + 1) * 128] for cc in range(NCC)]
            ws_v = [wvt[:, cc * 128:(cc + 1) * 128] for cc in range(NCC)]

            qkT = [[None, None], [None, None]]   # [q/k][tt] -> [128, 512] bf16

            def emit_qkv(tt, blk, eng):
                """one of q (blk 0) / k (blk 1) for query half tt."""
                ps = pst.tile([128, TT], F32, tag="st")
                w = ws_q if blk == 0 else ws_k
                for cc in range(NCC):
                    nc.tensor.matmul(ps[:], w[cc],
                                     xv[:, cc, tt * TT:(tt + 1) * TT],
                                     start=(cc == 0), stop=(cc == NCC - 1))
                qkT[blk][tt] = cp.tile([128, TT], BF16, tag=f"qk{blk}_{tt}",
                                       name=f"qk{blk}_{tt}")
                if eng == "act":
                    nc.scalar.activation(qkT[blk][tt][:], ps[:], AF.Identity,
                                         bias=bqk_sb[:, blk:blk + 1])
                else:
                    nc.vector.tensor_scalar_add(
                        qkT[blk][tt][:], ps[:], bqk_sb[:, blk:blk + 1])

            def emit_vt(tc_, eng="dve"):
                """V^T for key chunk tc_ via operand-swapped matmul; PSUM slot
                borrowed from the score pool."""
                pv = pst.tile([128, TT], F32, tag="st")
                for cc in range(NCC):
                    nc.tensor.matmul(
                        pv[:, 0:128], xv[:, cc, tc_ * 128:(tc_ + 1) * 128],
                        ws_v[cc], start=(cc == 0), stop=(cc == NCC - 1))
                dst = vaug[tc_][:, 0:130].rearrange(
                    "p (g c) -> p g c", c=65)[:, :, 0:64]
                src = pv[:, 0:128].rearrange("p (g c) -> p g c", g=2)
                if eng == "act":
                    nc.scalar.activation(dst, src, AF.Copy)
                else:
                    nc.vector.tensor_copy(dst, src)

            po = [None, None]

            def attn_chunk(qt, kc, mask_pool=False):
                """One key chunk vs query tile qt, both heads.

                Masks only touch the staircase region (first 128 or 256
                columns); columns right of it are fully visible, so the
                chain-b AV of split chunks never waits on a mask."""
                kcr = kc - qt * 4
                half = kcr >= 2                # upper diagonal: queries 256:512
                qoff, qlen = (HT, HT) if half else (0, TT)
                nlive = qt * 4 + 4
                pss, ees = [], []
                for hi in range(2):
                    ps = pst.tile([128, TT], F32, tag="st")
                    nc.tensor.matmul(
                        ps[:, 0:qlen],
                        qkT[1][kc // 4][hi * 64:(hi + 1) * 64,
                                        (kc % 4) * 128:(kc % 4) * 128 + 128],
                        qkT[0][qt][hi * 64:(hi + 1) * 64, qoff:qoff + qlen],
                        start=True, stop=True)
                    pss.append(ps)
                for hi in range(2):
                    e = ep.tile([128, TT], BF16, tag="e")
                    nc.scalar.activation(e[:, 0:qlen], pss[hi][:, 0:qlen],
                                         AF.Exp, scale=SCALE)
                    ees.append(e)
                masked = kcr >= 0
                mw = 128 * (kcr % 2 + 1)       # staircase width
                base = kcr - 2 if half else kcr

                def emit_mask(hi):
                    if not masked:
                        return
                    if mask_pool:
                        nc.gpsimd.affine_select(
                            ees[hi][:, 0:mw], ees[hi][:, 0:mw],
                            pattern=[[1, mw]],
                            compare_op=mybir.AluOpType.is_ge, fill=0.0,
                            base=-128 * base, channel_multiplier=-1)
                    else:
                        nc.vector.tensor_mul(ees[hi][:, 0:mw], ees[hi][:, 0:mw],
                                             masks[:, base * HT:base * HT + mw])

                # PSUM start=True zeroes the whole 2KB zero-region (the
                # full po row), so only kc0 carries start. Group flags: zone
                # opens at kc0, closes at the last full chunk; the halves
                # skip the check (values still accumulate).
                for hi in range(2):
                    emit_mask(hi)
                    nc.tensor.matmul(
                        po[hi][0:65, qoff:qoff + qlen],
                        vaug[kc][:, hi * 65:(hi + 1) * 65],
                        ees[hi][:, 0:qlen],
                        start=(kc == 0),
                        stop=(kc == nlive - 3 and not half),
                        skip_group_check=half)

            yT = [None, None]     # per qt: [128, 512] bf16 (both heads)

            def emit_norm(qt, cols, pot=None, pe_bcast=False):
                """softmax divide for column range cols of po -> yT[qt]."""
                po_ = pot if pot is not None else po
                o0, o1 = cols
                w = o1 - o0
                for hi in range(2):
                    rb = rbp.tile([1, TT], BF16, tag="rb")
                    with nc.allow_low_precision("recip feeds 2e-2-tol softmax"):
                        nc.vector.reciprocal(rb[:, 0:w], po_[hi][64:65, o0:o1])
                    if pe_bcast:
                        rp = pst.tile([128, TT], F32, tag="st")
                        nc.tensor.matmul(rp[0:64, 0:w], ones64[:],
                                         rb[0:1, 0:w], start=True, stop=True)
                        rbc = rp[0:64, 0:w]
                    else:
                        rbs = rbp.tile([64, TT], BF16, tag="rbc")
                        nc.gpsimd.partition_broadcast(rbs[:, 0:w], rb[0:1, 0:w])
                        rbc = rbs[:, 0:w]
                    nc.vector.tensor_mul(
                        yT[qt][hi * 64:(hi + 1) * 64, o0:o1],
                        po_[hi][0:64, o0:o1], rbc)

            ost = [cp.tile([128, 3 * TT], BF16, tag=f"ost{i}", name=f"ost{i}")
                   for i in range(4)]     # staging: (tt, et-half) -> 3 ets

            def emit_proj_et(tt, et, cols, eng, ppov=False):
                o0, o1 = cols
                w = o1 - o0
                if ppov:
                    pm = pov.tile([128, TT], F32, tag=f"po{et}",
                                  name=f"pmp{et}")
                else:
                    pm = pst.tile([128, TT], F32, tag="st")
                nc.tensor.matmul(pm[:, 0:w], wpt[:, et * 128:(et + 1) * 128],
                                 yT[tt][:, o0:o1], start=True, stop=True)
                st = ost[tt * 2 + et // 3]
                dst = st[:, (et % 3) * TT + o0:(et % 3) * TT + o1]
                if eng == "dve":
                    nc.vector.tensor_scalar_add(dst, pm[:, 0:w],
                                                bpr_sb[:, et:et + 1])
                else:
                    nc.scalar.activation(dst, pm[:, 0:w], AF.Identity,
                                         bias=bpr_sb[:, et:et + 1])

            def emit_out_dma(tt, ethalf, cols, sub=(0, 3)):
                o0, o1 = cols
                g0, g1 = sub
                st = ost[tt * 2 + ethalf]
                nc.sync.dma_start(
                    out=outT[ethalf * 384 + g0 * 128:ethalf * 384 + g1 * 128,
                             tt * TT + o0:tt * TT + o1]
                    .rearrange("(g p) t -> p g t", p=128),
                    in_=st[:].rearrange("p (g t) -> p g t", g=3)
                    [:, g0:g1, o0:o1])

            # ================= schedule =================
            emit_qkv(0, 0, "act")
            emit_qkv(0, 1, "act")
            emit_vt(0)
            emit_vt(1)

            po = [pov.tile([128, TT], F32, tag=f"po{hi}", name=f"po{hi}_a")
                  for hi in range(2)]
            attn_chunk(0, 0)
            emit_vt(2)
            emit_vt(3)
            attn_chunk(0, 1)
            emit_qkv(1, 0, "act")
            attn_chunk(0, 2)
            emit_qkv(1, 1, "act")
            attn_chunk(0, 3)
            yT[0] = cp.tile([128, TT], BF16, tag="y0", name="y0")
            po0 = po

            po = [pov.tile([128, TT], F32, tag=f"po{hi}", name=f"po{hi}_b")
                  for hi in range(2)]
            yT[1] = cp.tile([128, TT], BF16, tag="y1", name="y1")
            attn_chunk(1, 0)
            emit_vt(4, "act")
            emit_vt(5, "act")
            attn_chunk(1, 1)
            emit_norm(0, (0, TT), pot=po0)
            emit_vt(6, "act")
            emit_vt(7, "act")
            attn_chunk(1, 2)
            emit_proj_et(0, 0, (0, TT), "dve")
            attn_chunk(1, 3)
            emit_proj_et(0, 1, (0, TT), "dve")
            attn_chunk(1, 4)
            emit_proj_et(0, 2, (0, TT), "dve")
            emit_out_dma(0, 0, (0, TT))
            attn_chunk(1, 5)
            emit_norm(1, (0, HT))
            emit_proj_et(0, 3, (0, TT), "dve")
            attn_chunk(1, 6, mask_pool=True)
            emit_proj_et(0, 4, (0, TT), "dve")
            attn_chunk(1, 7, mask_pool=True)
            emit_proj_et(0, 5, (0, TT), "dve")
            emit_out_dma(0, 1, (0, TT))
            emit_norm(1, (HT, TT))
            emit_proj_et(1, 0, (0, HT), "act")
            emit_proj_et(1, 1, (0, HT), "dve")
            emit_proj_et(1, 2, (0, HT), "act")
            emit_out_dma(1, 0, (0, HT))
            emit_proj_et(1, 3, (0, HT), "dve")
            emit_proj_et(1, 4, (0, HT), "act")
            emit_proj_et(1, 5, (0, HT), "dve")
            emit_out_dma(1, 1, (0, HT))
            emit_proj_et(1, 0, (HT, TT), "act")
            emit_proj_et(1, 1, (HT, TT), "dve")
            emit_proj_et(1, 2, (HT, TT), "act")
            emit_out_dma(1, 0, (HT, TT))
            emit_proj_et(1, 3, (HT, TT), "dve")
            emit_proj_et(1, 4, (HT, TT), "dve")
            emit_proj_et(1, 5, (HT, TT), "act")
            emit_out_dma(1, 1, (HT, TT))
            _DBG["yT"] = yT
            _DBG["qkT"] = qkT
            _DBG["vaug"] = vaug
            _DBG["po1"] = po
    nc.compile()
    return nc


def _bf16(a):
    return np.ascontiguousarray(np.asarray(a, np.float32)).astype(
        ml_dtypes.bfloat16)


def _in_maps(x, W_attn, b_attn, W_proj, b_proj):
    xTn = _bf16(x.reshape(T, C).T)                       # [C, T]
    b_eff = (b_proj + b_attn[2 * C:] @ W_proj).astype(np.float32)
    maps = []
    for core in range(NCORES):
        h0, h1 = HEAD_MAP[core]
        hc = list(range(h0 * HS, (h0 + 1) * HS)) + \
            list(range(h1 * HS, (h1 + 1) * HS))
        wqc = W_attn[:, hc]                                           # [C, 128]
        wkc = W_attn[:, [C + i for i in hc]]                          # [C, 128]
        wvc = W_attn[:, [2 * C + i for i in hc]]                      # [C, 128]
        wpc = np.concatenate(
            [W_proj[h0 * HS:(h0 + 1) * HS, :],
             np.zeros_like(W_proj[:HS]) if h1 == h0
             else W_proj[h1 * HS:(h1 + 1) * HS, :]], axis=0)   # [128, C]
        bqkc = np.stack([np.concatenate([b_attn[p * C + h0 * HS:
                                                p * C + (h0 + 1) * HS],
                                         b_attn[p * C + h1 * HS:
                                                p * C + (h1 + 1) * HS]])
                         for p in range(2)], axis=1).astype(np.float32)
        bprc = (b_eff.reshape(NCC, 128).T if core == 0
                else np.zeros((128, NCC), np.float32))
        maps.append({
            "xT": xTn, "wq": _bf16(wqc), "wk": _bf16(wkc), "wv": _bf16(wvc),
            "wp": _bf16(wpc),
            "bqk": np.ascontiguousarray(bqkc),
            "bpr": np.ascontiguousarray(bprc),
        })
    return maps


def kernel(x, W_attn, b_attn, W_proj, b_proj, _trace=False, _trace_kwargs=None):
    x = np.asarray(x, np.float32)
    W_attn = np.asarray(W_attn, np.float32)
    b_attn = np.asarray(b_attn, np.float32)
    W_proj = np.asarray(W_proj, np.float32)
    b_proj = np.asarray(b_proj, np.float32)

    if "nc" not in _CACHE:
        _CACHE["nc"] = _build_program()
    nc = _CACHE["nc"]

    maps = _in_maps(x, W_attn, b_attn, W_proj, b_proj)
    kw = {}
    if _trace:
        kw = dict(trace=True, **(_trace_kwargs or {}))
    br = run_bass_kernel_spmd(nc, maps, list(range(NCORES)), **kw)
    acc = np.zeros((C, T), np.float64)
    for core in range(NCORES):
        acc += br.results[core]["outT"].astype(np.float64)
    out = np.ascontiguousarray(acc.T.astype(np.float32)).reshape(1, T, C)
    _CACHE["last_results"] = br
    return out


# revision 57
# speedup vs baseline: 1.0055x; 1.0055x over previous
"""TRN2 Bass kernel for nn_CausalSelfAttention_4054449128214.

The reference returns out_s + stop_gradient(out_full - out_s), whose forward
value is exactly out_full -- plain dense causal self-attention. So the kernel
computes: qkv = x@W_attn+b_attn, per-head causal softmax attention,
y@W_proj+b_proj.

Sharding (8 cores, no collectives): Megatron head-parallel. Cores 0-3 own head
pairs (0,1)..(6,7); cores 4-7 own heads 8..11 (duplicated for SPMD
shape-uniformity, second W_proj slice zeroed). Each core computes its heads'
QKV columns, attention, and a row-sliced partial output projection; the host
sums the 8 partials (the Megatron row-parallel all-reduce) and transposes.

Design (36.7us cost-model time vs 46.2us f32r baseline):
- everything bf16: halves DMA, full-rate matmuls at any free size, 2x DVE
  modes. End-to-end rel err ~4e-3 (gate 2e-2).
- V is produced directly transposed ([token, channel]) by operand-swapped
  matmuls (stationary = x chunk, moving = W_v); the softmax denominator rides
  as an all-ones 65th column of the V^T tiles, so it falls out of the AV
  matmul for free.
- causal skip: upper-diagonal key chunks run on half (256-query) tiles, and
  mask multiplies only touch the staircase region (<=256 cols) of each e
  tile; fully-visible columns never wait on a mask.
- output projection uses contraction 128 (both heads in one matmul);
  queries 512:768 are normalized/projected as soon as key chunk 5 lands
  (the later chunks only touch columns 256:512 of the PSUM accumulator).
- engine roles: Act = exp stream (+ the PSUM->SBUF copies that gate it,
  which its in-order queue naturally sequences); DVE = masks, normalize,
  most proj copies; Pool = partition broadcasts, late-half affine masks,
  memsets; PE warms its p-state on a dummy chain during the input DMAs.
- PSUM: one 6-deep [128,512] ring (qkv/scores/vT/proj) + 2 banks for the
  two per-head [65,512] AV accumulators.
- PSUM start=True zeroes a whole 2KB zero-region, DMA cannot touch PSUM,
  GPSIMD cannot touch PSUM, and TensorTensor may read only one PSUM
  operand -- all load-bearing constraints below.
"""

import numpy as np
import ml_dtypes

import concourse.bacc as bacc
import concourse.mybir as mybir
import concourse.tile as tile
from concourse.bass_utils import run_bass_kernel_spmd

F32 = mybir.dt.float32
BF16 = mybir.dt.bfloat16
AF = mybir.ActivationFunctionType

T = 1024          # sequence length
C = 768           # channels
NH = 12           # heads
HS = 64           # head size
NCORES = 8
TT = 512          # query tile
HT = 256          # half tile
NCC = C // 128    # 6 contraction chunks
NKC = T // 128    # 8 key chunks
SCALE = 1.0 / 8.0  # 1/sqrt(HS)

# core -> (head0, head1); cores 4-7 duplicate their head (2nd W_proj slice zeroed)
HEAD_MAP = [(0, 1), (2, 3), (4, 5), (6, 7), (8, 8), (9, 9), (10, 10), (11, 11)]

_CACHE: dict = {}
_DBG: dict = {}


def _build_program():
    nc = bacc.Bacc("TRN2", target_bir_lowering=False, debug=False,
                   num_devices=NCORES)
    xT = nc.dram_tensor("xT", [C, T], BF16, kind="ExternalInput").ap()
    wq = nc.dram_tensor("wq", [C, 128], BF16, kind="ExternalInput").ap()
    wkv = nc.dram_tensor("wkv", [C, 256], BF16, kind="ExternalInput").ap()
    wp = nc.dram_tensor("wp", [128, C], BF16, kind="ExternalInput").ap()
    bqk = nc.dram_tensor("bqk", [128, 2], F32, kind="ExternalInput").ap()
    bpr = nc.dram_tensor("bpr", [128, NCC], F32, kind="ExternalInput").ap()
    outT = nc.dram_tensor("outT", [C, T], BF16, kind="ExternalOutput").ap()

    with tile.TileContext(nc) as tc:
        with (
            tc.tile_pool(name="const", bufs=1) as cp,
            tc.tile_pool(name="e", bufs=8) as ep,
            tc.tile_pool(name="rb", bufs=2) as rbp,
            tc.tile_pool(name="pst", bufs=4, space="PSUM") as pst,
            tc.tile_pool(name="pov", bufs=2, space="PSUM") as pov,
        ):
            # preload the Act function table (exp_and_others) and ramp the
            # PE p-state off the critical path while DMAs run
            tiny = cp.tile([1, 128], BF16, tag="tiny")
            nc.vector.memset(tiny[:], 1.0)
            nc.scalar.activation(tiny[:, 64:65], tiny[:, 0:1], AF.Exp)
            pwu = pst.tile([128, TT], F32, tag="st")
            for _ in range(45):
                nc.tensor.matmul(pwu[0:1, 0:64], tiny[:, 0:1], tiny[:, 0:64],
                                 start=True, stop=True)

            # ---- input DMAs (SP queue), in need order
            wqt = cp.tile([128, NCC * 128], BF16, tag="wqt")
            nc.sync.dma_start(
                out=wqt[:].rearrange("p (c j) -> p c j", c=NCC),
                in_=wq.rearrange("(c p) j -> p c j", p=128))
            xt = cp.tile([128, NCC * T], BF16, tag="xt")
            x3 = xT.rearrange("(c p) t -> p c t", p=128)
            xv = xt[:].rearrange("p (c t) -> p c t", c=NCC)
            for c0 in (0, 2):             # first 512 tokens, cc pairs
                nc.sync.dma_start(out=xv[:, c0:c0 + 2, 0:TT],
                                  in_=x3[:, c0:c0 + 2, 0:TT])
            wkvt = cp.tile([128, NCC * 256], BF16, tag="wkvt")
            nc.sync.dma_start(
                out=wkvt[:].rearrange("p (c j) -> p c j", c=NCC),
                in_=wkv.rearrange("(c p) j -> p c j", p=128))
            nc.sync.dma_start(out=xv[:, 4:6, 0:TT], in_=x3[:, 4:6, 0:TT])
            for c0 in (0, 2, 4):          # second 512 tokens
                nc.sync.dma_start(out=xv[:, c0:c0 + 2, TT:T],
                                  in_=x3[:, c0:c0 + 2, TT:T])
            bpr_sb = cp.tile([128, NCC], F32, tag="bpr")
            nc.sync.dma_start(out=bpr_sb[:], in_=bpr)
            wpt = cp.tile([128, C], BF16, tag="wpt")
            nc.sync.dma_start(out=wpt[:], in_=wp)

            # tiny early constant on Pool's DMA queue
            bqk_sb = cp.tile([128, 2], F32, tag="bqk")
            nc.gpsimd.dma_start(out=bqk_sb[:], in_=bqk)


            ones64 = cp.tile([1, 64], BF16, tag="ones64")
            nc.gpsimd.memset(ones64[:], 1.0)

            # causal staircase masks, generated on the idle early Pool engine:
            # cols 0:256 keep q>=p (M0), cols 256:512 keep q>=p+128 (M1)
            masks = cp.tile([128, TT], BF16, tag="masks")
            nc.gpsimd.memset(masks[:], 1.0)
            nc.gpsimd.affine_select(
                masks[:, 0:HT], masks[:, 0:HT], pattern=[[1, HT]],
                compare_op=mybir.AluOpType.is_ge, fill=0.0,
                base=0, channel_multiplier=-1)
            nc.gpsimd.affine_select(
                masks[:, HT:TT], masks[:, HT:TT], pattern=[[1, HT]],
                compare_op=mybir.AluOpType.is_ge, fill=0.0,
                base=-128, channel_multiplier=-1)

            # V^T tiles: [key, 2*(64 ch + ones col)]; ones preset via memset
            vaug = [cp.tile([128, 130], BF16, tag=f"va{kc}", name=f"va{kc}")
                    for kc in range(NKC)]
            for kc in range(NKC):
                nc.gpsimd.memset(vaug[kc][:], 1.0)

            ws_q = [wqt[:, cc * 128:(cc + 1) * 128] for cc in range(NCC)]
            ws_k = [wkvt[:, cc * 256:cc * 256 + 128] for cc in range(NCC)]
            ws_v = [wkvt[:, cc * 256 + 128:cc * 256 + 256] for cc in range(NCC)]

            qkT = [[None, None], [None, None]]   # [q/k][tt] -> [128, 512] bf16

            def emit_qkv(tt, blk, eng):
                """one of q (blk 0) / k (blk 1) for query half tt."""
                ps = pst.tile([128, TT], F32, tag="st")
                w = ws_q if blk == 0 else ws_k
                for cc in range(NCC):
                    nc.tensor.matmul(ps[:], w[cc],
                                     xv[:, cc, tt * TT:(tt + 1) * TT],
                                     start=(cc == 0), stop=(cc == NCC - 1))
                qkT[blk][tt] = cp.tile([128, TT], BF16, tag=f"qk{blk}_{tt}",
                                       name=f"qk{blk}_{tt}")
                if eng == "act":
                    nc.scalar.activation(qkT[blk][tt][:], ps[:], AF.Identity,
                                         bias=bqk_sb[:, blk:blk + 1])
                else:
                    nc.vector.tensor_scalar_add(
                        qkT[blk][tt][:], ps[:], bqk_sb[:, blk:blk + 1])

            def emit_vt(tc_, eng="dve"):
                """V^T for key chunk tc_ via operand-swapped matmul; PSUM slot
                borrowed from the score pool."""
                pv = pst.tile([128, TT], F32, tag="st")
                for cc in range(NCC):
                    nc.tensor.matmul(
                        pv[:, 0:128], xv[:, cc, tc_ * 128:(tc_ + 1) * 128],
                        ws_v[cc], start=(cc == 0), stop=(cc == NCC - 1))
                dst = vaug[tc_][:, 0:130].rearrange(
                    "p (g c) -> p g c", c=65)[:, :, 0:64]
                src = pv[:, 0:128].rearrange("p (g c) -> p g c", g=2)
                if eng == "act":
                    nc.scalar.activation(dst, src, AF.Copy)
                else:
                    nc.vector.tensor_copy(dst, src)

            po = [None, None]

            def attn_chunk(qt, kc, mask_pool=False):
                """One key chunk vs query tile qt, both heads.

                Masks only touch the staircase region (first 128 or 256
                columns); columns right of it are fully visible, so the
                chain-b AV of split chunks never waits on a mask."""
                kcr = kc - qt * 4
                half = kcr >= 2                # upper diagonal: queries 256:512
                qoff, qlen = (HT, HT) if half else (0, TT)
                nlive = qt * 4 + 4
                pss, ees = [], []
                for hi in range(2):
                    ps = pst.tile([128, TT], F32, tag="st")
                    nc.tensor.matmul(
                        ps[:, 0:qlen],
                        qkT[1][kc // 4][hi * 64:(hi + 1) * 64,
                                        (kc % 4) * 128:(kc % 4) * 128 + 128],
                        qkT[0][qt][hi * 64:(hi + 1) * 64, qoff:qoff + qlen],
                        start=True, stop=True)
                    pss.append(ps)
                for hi in range(2):
                    e = ep.tile([128, TT], BF16, tag="e")
                    nc.scalar.activation(e[:, 0:qlen], pss[hi][:, 0:qlen],
                                         AF.Exp, scale=SCALE)
                    ees.append(e)
                masked = kcr >= 0
                mw = 128 * (kcr % 2 + 1)       # staircase width
                base = kcr - 2 if half else kcr

                def emit_mask(hi):
                    if not masked:
                        return
                    if mask_pool:
                        nc.gpsimd.affine_select(
                            ees[hi][:, 0:mw], ees[hi][:, 0:mw],
                            pattern=[[1, mw]],
                            compare_op=mybir.AluOpType.is_ge, fill=0.0,
                            base=-128 * base, channel_multiplier=-1)
                    else:
                        nc.vector.tensor_mul(ees[hi][:, 0:mw], ees[hi][:, 0:mw],
                                             masks[:, base * HT:base * HT + mw])

                # PSUM start=True zeroes the whole 2KB zero-region (the
                # full po row), so only kc0 carries start. Group flags: zone
                # opens at kc0, closes at the last full chunk; the halves
                # skip the check (values still accumulate).
                for hi in range(2):
                    emit_mask(hi)
                    nc.tensor.matmul(
                        po[hi][0:65, qoff:qoff + qlen],
                        vaug[kc][:, hi * 65:(hi + 1) * 65],
                        ees[hi][:, 0:qlen],
                        start=(kc == 0),
                        stop=(kc == nlive - 3 and not half),
                        skip_group_check=half)

            yT = [None, None]     # per qt: [128, 512] bf16 (both heads)

            def emit_norm(qt, cols, pot=None, pe_bcast=False):
                """softmax divide for column range cols of po -> yT[qt]."""
                po_ = pot if pot is not None else po
                o0, o1 = cols
                w = o1 - o0
                for hi in range(2):
                    rb = rbp.tile([1, TT], BF16, tag="rb")
                    with nc.allow_low_precision("recip feeds 2e-2-tol softmax"):
                        nc.vector.reciprocal(rb[:, 0:w], po_[hi][64:65, o0:o1])
                    if pe_bcast:
                        rp = pst.tile([128, TT], F32, tag="st")
                        nc.tensor.matmul(rp[0:64, 0:w], ones64[:],
                                         rb[0:1, 0:w], start=True, stop=True)
                        rbc = rp[0:64, 0:w]
                    else:
                        rbs = rbp.tile([64, TT], BF16, tag="rbc")
                        nc.gpsimd.partition_broadcast(rbs[:, 0:w], rb[0:1, 0:w])
                        rbc = rbs[:, 0:w]
                    nc.vector.tensor_mul(
                        yT[qt][hi * 64:(hi + 1) * 64, o0:o1],
                        po_[hi][0:64, o0:o1], rbc)

            ost = [cp.tile([128, 3 * TT], BF16, tag=f"ost{i}", name=f"ost{i}")
                   for i in range(4)]     # staging: (tt, et-half) -> 3 ets

            def emit_proj_et(tt, et, cols, eng, ppov=False):
                o0, o1 = cols
                w = o1 - o0
                if ppov:
                    pm = pov.tile([128, TT], F32, tag=f"po{et}",
                                  name=f"pmp{et}")
                else:
                    pm = pst.tile([128, TT], F32, tag="st")
                nc.tensor.matmul(pm[:, 0:w], wpt[:, et * 128:(et + 1) * 128],
                                 yT[tt][:, o0:o1], start=True, stop=True)
                st = ost[tt * 2 + et // 3]
                dst = st[:, (et % 3) * TT + o0:(et % 3) * TT + o1]
                if eng == "dve":
                    nc.vector.tensor_scalar_add(dst, pm[:, 0:w],
                                                bpr_sb[:, et:et + 1])
                else:
                    nc.scalar.activation(dst, pm[:, 0:w], AF.Identity,
                                         bias=bpr_sb[:, et:et + 1])

            def emit_out_dma(tt, ethalf, cols, sub=(0, 3)):
                o0, o1 = cols
                g0, g1 = sub
                st = ost[tt * 2 + ethalf]
                nc.sync.dma_start(
                    out=outT[ethalf * 384 + g0 * 128:ethalf * 384 + g1 * 128,
                             tt * TT + o0:tt * TT + o1]
                    .rearrange("(g p) t -> p g t", p=128),
                    in_=st[:].rearrange("p (g t) -> p g t", g=3)
                    [:, g0:g1, o0:o1])

            # ================= schedule =================
            emit_qkv(0, 0, "act")
            emit_qkv(0, 1, "act")
            emit_vt(0)
            emit_vt(1)

            po = [pov.tile([128, TT], F32, tag=f"po{hi}", name=f"po{hi}_a")
                  for hi in range(2)]
            attn_chunk(0, 0)
            emit_vt(2)
            emit_vt(3)
            attn_chunk(0, 1)
            emit_qkv(1, 0, "act")
            attn_chunk(0, 2)
            emit_qkv(1, 1, "act")
            attn_chunk(0, 3)
            yT[0] = cp.tile([128, TT], BF16, tag="y0", name="y0")
            po0 = po

            po = [pov.tile([128, TT], F32, tag=f"po{hi}", name=f"po{hi}_b")
                  for hi in range(2)]
            yT[1] = cp.tile([128, TT], BF16, tag="y1", name="y1")
            attn_chunk(1, 0)
            emit_vt(4, "act")
            emit_vt(5, "act")
            attn_chunk(1, 1)
            emit_norm(0, (0, TT), pot=po0)
            emit_vt(6, "act")
            emit_vt(7, "act")
            attn_chunk(1, 2)
            emit_proj_et(0, 0, (0, TT), "dve")
            attn_chunk(1, 3)
            emit_proj_et(0, 1, (0, TT), "dve")
            attn_chunk(1, 4)
            emit_proj_et(0, 2, (0, TT), "dve")
            emit_out_dma(0, 0, (0, TT))
            attn_chunk(1, 5)
            emit_norm(1, (0, HT))
            emit_proj_et(0, 3, (0, TT), "dve")
            attn_chunk(1, 6, mask_pool=True)
            emit_proj_et(0, 4, (0, TT), "dve")
            attn_chunk(1, 7, mask_pool=True)
            emit_proj_et(0, 5, (0, TT), "dve")
            emit_out_dma(0, 1, (0, TT))
            emit_norm(1, (HT, TT))
            emit_proj_et(1, 0, (0, HT), "act")
            emit_proj_et(1, 1, (0, HT), "dve")
            emit_proj_et(1, 2, (0, HT), "act")
            emit_out_dma(1, 0, (0, HT))
            emit_proj_et(1, 3, (0, HT), "dve")
            emit_proj_et(1, 4, (0, HT), "act")
            emit_proj_et(1, 5, (0, HT), "dve")
            emit_out_dma(1, 1, (0, HT))
            emit_proj_et(1, 0, (HT, TT), "act")
            emit_proj_et(1, 1, (HT, TT), "dve")
            emit_proj_et(1, 2, (HT, TT), "act")
            emit_out_dma(1, 0, (HT, TT))
            emit_proj_et(1, 3, (HT, TT), "dve")
            emit_proj_et(1, 4, (HT, TT), "dve")
            emit_proj_et(1, 5, (HT, TT), "act")
            emit_out_dma(1, 1, (HT, TT))
            _DBG["yT"] = yT
            _DBG["qkT"] = qkT
            _DBG["vaug"] = vaug
            _DBG["po1"] = po
    nc.compile()
    return nc


def _bf16(a):
    return np.ascontiguousarray(np.asarray(a, np.float32)).astype(
        ml_dtypes.bfloat16)


def _in_maps(x, W_attn, b_attn, W_proj, b_proj):
    xTn = _bf16(x.reshape(T, C).T)                       # [C, T]
    b_eff = (b_proj + b_attn[2 * C:] @ W_proj).astype(np.float32)
    maps = []
    for core in range(NCORES):
        h0, h1 = HEAD_MAP[core]
        hc = list(range(h0 * HS, (h0 + 1) * HS)) + \
            list(range(h1 * HS, (h1 + 1) * HS))
        wqc = W_attn[:, hc]                                           # [C, 128]
        wkvc = np.concatenate(
            [W_attn[:, [C + i for i in hc]],
             W_attn[:, [2 * C + i for i in hc]]], axis=1)             # [C, 256]
        wpc = np.concatenate(
            [W_proj[h0 * HS:(h0 + 1) * HS, :],
             np.zeros_like(W_proj[:HS]) if h1 == h0
             else W_proj[h1 * HS:(h1 + 1) * HS, :]], axis=0)   # [128, C]
        bqkc = np.stack([np.concatenate([b_attn[p * C + h0 * HS:
                                                p * C + (h0 + 1) * HS],
                                         b_attn[p * C + h1 * HS:
                                                p * C + (h1 + 1) * HS]])
                         for p in range(2)], axis=1).astype(np.float32)
        bprc = (b_eff.reshape(NCC, 128).T if core == 0
                else np.zeros((128, NCC), np.float32))
        maps.append({
            "xT": xTn, "wq": _bf16(wqc), "wkv": _bf16(wkvc),
            "wp": _bf16(wpc),
            "bqk": np.ascontiguousarray(bqkc),
            "bpr": np.ascontiguousarray(bprc),
        })
    return maps


def kernel(x, W_attn, b_attn, W_proj, b_proj, _trace=False, _trace_kwargs=None):
    x = np.asarray(x, np.float32)
    W_attn = np.asarray(W_attn, np.float32)
    b_attn = np.asarray(b_attn, np.float32)
    W_proj = np.asarray(W_proj, np.float32)
    b_proj = np.asarray(b_proj, np.float32)

    if "nc" not in _CACHE:
        _CACHE["nc"] = _build_program()
    nc = _CACHE["nc"]

    maps = _in_maps(x, W_attn, b_attn, W_proj, b_proj)
    kw = {}
    if _trace:
        kw = dict(trace=True, **(_trace_kwargs or {}))
    br = run_bass_kernel_spmd(nc, maps, list(range(NCORES)), **kw)
    acc = np.zeros((C, T), np.float64)
    for core in range(NCORES):
        acc += br.results[core]["outT"].astype(np.float64)
    out = np.ascontiguousarray(acc.T.astype(np.float32)).reshape(1, T, C)
    _CACHE["last_results"] = br
    return out
